# revision 24
# baseline (speedup 1.0000x reference)
"""Trainium2 Bass kernel for nn_CompositionalNN_17308718202922.

Math: the reference only uses timestep 0 of both LSTM directions
(yf[0], yr[0]), and both directions consume the same first input
x[-1].  So the whole 64-step recurrence collapses to a single LSTM
cell step per direction plus the final linear+tanh:

    g_d   = nt @ Wih_d.T + h0_d @ Whh_d.T + bih_d + bhh_d   (d in {f, r})
    h_d   = sigmoid(o) * tanh(sigmoid(f) * c0_d + sigmoid(i) * tanh(g))
    out   = tanh([h_f, h_r] @ W_lin.T + b_lin)

Sharding (8 cores): tensor-parallel over the hidden dim.  Core k
computes hidden units [256k, 256k+256) of both directions, then the
final linear is contraction-sharded: each core multiplies its local
512-long h slice against its (512, 2048) W_lin rows and a
ReduceScatter(+add) yields each core's 256-long output slice (bias is
folded into rank 0's matmul; tanh applied post-RS on device).

Mixed-precision weight streaming (DMA is the roofline):
  - x-side Wih chunks: 4/16 in bf16 + 12/16 in fp8-e3m4 (x2^7 scale)
  - h-side Whh: fp8-e5m2 (h0 term is ~30x smaller than the x term, so
    e5m2's 2 mantissa bits suffice), fed via DoubleRow pairs (2
    k-chunks per PE instruction); h0 lhsT in e5m2 (x2^6)
  - W_lin: bf16 (last-layer error does not average out)
  - psum carries 2^7-scaled gates; the activation's scale=2^-7
    descales for free; gate biases ride a K=2 mini-matmul.
Per-core HBM traffic ~11.1 MiB vs 18.5 MiB for all-bf16.
"""

import numpy as np
import ml_dtypes

import concourse.tile as tile
from concourse import bacc, mybir
from concourse.bass_utils import run_bass_kernel_spmd

H = 2048
NC = 8
HS = H // NC          # 256 hidden units per core per direction
G = 4 * HS            # 1024 gate cols per core per direction [i,f,o,g]
NXC = 16              # x-side k-chunks (2048 / 128)
NXB = 0               # x-side chunks kept in bf16 (0: all e3m4)
NXQ = NXC - NXB       # x-side chunks in fp8-e3m4
NHP = 8               # h-side DoubleRow k-chunk pairs (16 chunks)
SB = 7                # log2 weight scale for the gate matmul
SH5 = 6               # log2 scale of the h0 lhsT (e5m2)
REV = 17               # bumps the zz input width: distinct HLO per program rev

BF = ml_dtypes.bfloat16
E3 = ml_dtypes.float8_e3m4
E5 = ml_dtypes.float8_e5m2

_CACHE: dict = {}


def _build_program(variant: str = "full"):
    f32 = mybir.dt.float32
    bf = mybir.dt.bfloat16
    e3 = mybir.dt.float8e3
    e5 = mybir.dt.float8e5
    nc = bacc.Bacc("TRN2", target_bir_lowering=False, debug=False, num_devices=NC)

    # --- DRAM inputs (all in final SBUF tile layout) ---
    sb1 = nc.dram_tensor("sb1", [128, NXC, 2, 2], bf, kind="ExternalInput")
    # sb2 (2 partitions; matmul lhsT/rhs must share base partition 0):
    #   cols 0:2 I2; 2:1026 gate-bias rhs; 1026:1282 c0;
    #   1282:3330 final-bias rhs (row 1 zeros)
    sb2 = nc.dram_tensor("sb2", [2, 3336], bf, kind="ExternalInput")
    # DoubleRow LDWEIGHTS needs the k-tile step %16B == 0: pad cols to 16
    h5 = nc.dram_tensor("h5", [128, 2, NHP, 2, 16], e5, kind="ExternalInput")
    wxb = ([nc.dram_tensor(f"wxb{d}", [128, NXB, G], bf, kind="ExternalInput")
            for d in range(2)] if NXB else None)
    wxq = [nc.dram_tensor(f"wxq{d}", [128, NXQ, G], e3, kind="ExternalInput")
           for d in range(2)]
    wh = [nc.dram_tensor(f"wh{d}", [128, NHP, 2, G], e5, kind="ExternalInput")
          for d in range(2)]
    wl = nc.dram_tensor("wl", [128, 4, H], bf, kind="ExternalInput")
    zz = nc.dram_tensor("zz", [1, REV], f32, kind="ExternalInput")
    if variant == "gates":
        out = nc.dram_tensor("out", [2, HS], f32, kind="ExternalOutput")
    else:
        out = nc.dram_tensor("out", [1, HS], f32, kind="ExternalOutput")

    SIG = mybir.ActivationFunctionType.Sigmoid
    TANH = mybir.ActivationFunctionType.Tanh
    DR = mybir.MatmulPerfMode.DoubleRow
    DS = float(2.0 ** -SB)

    with tile.TileContext(nc) as tc:
        with (
            tc.tile_pool(name="wpool", bufs=1) as wpool,
            tc.tile_pool(name="const", bufs=1) as const,
            tc.tile_pool(name="work", bufs=1) as work,
            tc.tile_pool(name="psum", bufs=1, space="PSUM") as psum,
            tc.tile_pool(name="dram", bufs=1, space="DRAM") as dram,
        ):
            # --- first weight DMA leads; smalls right behind ---
            # one tile per DMA: Tile deps are per-tile, so matmuls can
            # start as soon as their own chunk group lands
            if NXB:
                wxb0_t = wpool.tile([128, NXB, G], bf)
                nc.sync.dma_start(out=wxb0_t[:], in_=wxb[0].ap())
            else:
                wxq00_t = wpool.tile([128, 2, G], e3)
                nc.sync.dma_start(out=wxq00_t[:], in_=wxq[0].ap()[:, 0:2, :])
            sb1_t = const.tile([128, NXC, 2, 2], bf)
            nc.sync.dma_start(out=sb1_t[:], in_=sb1.ap())
            sb2_t = const.tile([2, 3336], bf)
            nc.sync.dma_start(out=sb2_t[:], in_=sb2.ap())
            h5_t = const.tile([128, 2, NHP, 2, 16], e5)
            nc.sync.dma_start(out=h5_t[:], in_=h5.ap())

            # --- preload activation tables off the critical path ---
            scr = work.tile([1, 2], f32)
            nc.scalar.activation(scr[:, 0:1], sb2_t[0:1, 2:3], SIG)
            nc.scalar.activation(scr[:, 1:2], sb2_t[0:1, 2:3], TANH)
            zf2 = work.tile([2, 1], f32)
            nc.vector.tensor_sub(zf2[:], sb2_t[:, 0:1], sb2_t[:, 0:1])

            # --- psum tiles ---
            psum_g0 = psum.tile([2, 512], f32)      # [i|f] x {fwd,rev}
            psum_g1 = psum.tile([2, 512], f32)      # [o|g] x {fwd,rev}
            psum_o = [psum.tile([1, 512], f32, name=f"psum_o{i}")
                      for i in range(4)]

            # --- bias mini-matmuls open the psum accumulation groups ---
            i2 = sb2_t[:, 0:2]
            nc.tensor.matmul(psum_g0[:], i2, sb2_t[:, 2:514],
                             start=True, stop=False)
            nc.tensor.matmul(psum_g1[:], i2, sb2_t[:, 514:1026],
                             start=True, stop=False)
            for nt in range(4):
                nc.tensor.matmul(
                    psum_o[nt][:], sb2_t[:, 0:1],
                    sb2_t[:, 1282 + 512 * nt:1282 + 512 * (nt + 1)],
                    start=True, stop=False,
                )

            # --- gate matmuls: per direction, 8 groups of [2 e3m4 x-chunks,
            # 1 e5m2 h-pair].  PE work per group (852+214 ns) < DMA cadence
            # (2x728 ns), keeping the PE dispatch-gated at full pstate ---
            for d in range(2):
                # DMA order: interleave [x-pair, h-pair] but pull the last
                # two x DMAs forward so the stream ENDS on PE-light h-pairs
                # (the PE backlog drains before the elementwise chain)
                seq = []
                for j in range(NXQ // 2):
                    seq += [("q", j), ("h", j)]
                seq = (seq[:-4] + ([("b", 0)] if NXB else []) +
                       [("q", NXQ // 2 - 2), ("h", NHP - 2),
                        ("q", NXQ // 2 - 1), ("h", NHP - 1)])
                for kind, j in seq:
                    if kind == "q":
                        if d == 0 and j == 0 and not NXB:
                            q2 = wxq00_t
                        else:
                            q2 = wpool.tile([128, 2, G], e3, name=f"wxq{d}_{j}")
                            nc.sync.dma_start(
                                out=q2[:],
                                in_=wxq[d].ap()[:, 2 * j:2 * j + 2, :])
                        for cc in range(2):
                            lhsT = sb1_t[:, NXB + 2 * j + cc, :, d]
                            nc.tensor.matmul(psum_g0[:], lhsT, q2[:, cc, 0:512],
                                             start=False, stop=False)
                            nc.tensor.matmul(psum_g1[:], lhsT, q2[:, cc, 512:1024],
                                             start=False, stop=False)
                    elif kind == "b":
                        if d == 0:
                            bt = wxb0_t
                        else:
                            bt = wpool.tile([128, NXB, G], bf, name="wxb1_t")
                            nc.sync.dma_start(out=bt[:], in_=wxb[d].ap())
                        for c in range(NXB):
                            lhsT = sb1_t[:, c, :, d]
                            nc.tensor.matmul(psum_g0[:], lhsT, bt[:, c, 0:512],
                                             start=False, stop=False)
                            nc.tensor.matmul(psum_g1[:], lhsT, bt[:, c, 512:1024],
                                             start=False, stop=False)
                    else:
                        h2 = wpool.tile([128, 1, 2, G], e5, name=f"wh{d}_{j}")
                        nc.sync.dma_start(out=h2[:],
                                          in_=wh[d].ap()[:, j:j + 1, :, :])
                        last = (d == 1) and (j == NHP - 1)
                        lhsT = h5_t[:, d, j, :, 0:2]
                        nc.tensor.matmul(psum_g0[:], lhsT, h2[:, 0, :, 0:512],
                                         start=False, stop=last, perf_mode=DR)
                        nc.tensor.matmul(psum_g1[:], lhsT, h2[:, 0, :, 512:1024],
                                         start=False, stop=last, perf_mode=DR)

            # --- W_lin slice last: hidden under the elementwise tail ---
            wl_t = []
            for s in range(4):
                w1 = wpool.tile([128, H], bf, name=f"wl_{s}")
                nc.sync.dma_start(out=w1[:], in_=wl.ap()[:, s, :])
                wl_t.append(w1)


            # --- LSTM cell elementwise (both dirs on 2 partitions) ---
            t_g = work.tile([2, HS], f32)
            nc.scalar.activation(t_g[:], psum_g1[:, 256:512], TANH, scale=DS)
            s_if = work.tile([2, 512], f32)
            nc.scalar.activation(s_if[:], psum_g0[:], SIG, scale=DS)
            s_o = work.tile([2, HS], f32)
            nc.scalar.activation(s_o[:], psum_g1[:, 0:256], SIG, scale=DS)

            # PE warmup: zero-adding f32 matmuls gated on the elementwise
            # chain (rhs = s_if) hold full pstate until the final matmuls.
            for i in range(2):
                nc.tensor.matmul(psum_o[i % 4][:], zf2[:, 0:1], s_if[:],
                                 start=False, stop=False)
            t1 = work.tile([2, HS], f32)
            nc.vector.tensor_mul(t1[:], s_if[:, 0:256], t_g[:])        # i*g
            t2 = work.tile([2, HS], f32)
            nc.vector.tensor_mul(t2[:], s_if[:, 256:512], sb2_t[:, 1026:1282])
            cn = work.tile([2, HS], f32)
            nc.vector.tensor_add(cn[:], t1[:], t2[:])
            tc_ = work.tile([2, HS], f32)
            nc.scalar.activation(tc_[:], cn[:], TANH)
            hh = work.tile([2, HS], bf)
            nc.vector.tensor_mul(hh[:], s_o[:], tc_[:])                # h

            if variant == "gates":
                hh32 = work.tile([2, HS], f32)
                nc.vector.tensor_copy(hh32[:], hh[:])
                nc.sync.dma_start(out=out.ap(), in_=hh32[:])
            else:
                # --- transpose h on the PE: (2,256) -> (128, 2, 2) ---
                i2b = sb2_t[:, 0:2]
                pt0 = psum.tile([128, 2], bf)
                pt1 = psum.tile([128, 2], bf)
                nc.tensor.transpose(pt0[:], hh[:, 0:128], i2b)
                nc.tensor.transpose(pt1[:], hh[:, 128:256], i2b)
                hT = work.tile([128, 2, 2], bf)
                nc.vector.tensor_copy(hT[:, 0, :], pt0[:])
                nc.vector.tensor_copy(hT[:, 1, :], pt1[:])

                # --- final linear partials: kc-outer matches wl arrival ---
                # k-chunks: 0 = h_f[0:128], 1 = h_f[128:256],
                #           2 = h_r[0:128], 3 = h_r[128:256]
                for kc in range(4):
                    lhsT = hT[:, kc % 2, kc // 2:kc // 2 + 1]  # (128, 1)
                    for nt in range(4):
                        nc.tensor.matmul(
                            psum_o[nt][:], lhsT,
                            wl_t[kc][:, 512 * nt:512 * (nt + 1)],
                            start=False, stop=(kc == 3),
                        )

                # --- psum -> sbuf (split over DVE+Act), po DMAs pipelined ---
                COPY = mybir.ActivationFunctionType.Copy
                pvec = work.tile([1, H], f32)
                po = dram.tile([1, H], mybir.dt.float32)
                rso = dram.tile([1, HS], mybir.dt.float32)
                for nt in range(4):
                    sl_ = slice(512 * nt, 512 * (nt + 1))
                    if nt % 2 == 0:
                        nc.vector.tensor_copy(pvec[:, sl_], psum_o[nt][:])
                    else:
                        nc.scalar.activation(pvec[:, sl_], psum_o[nt][:], COPY)
                nc.sync.dma_start(out=po[:], in_=pvec[:])
                if variant == "timing":
                    # collective-free twin for TimelineSim cost modelling
                    nc.sync.dma_start(out=rso[:], in_=po[:, 0:HS])
                else:
                    nc.gpsimd.collective_compute(
                        "ReduceScatter",
                        mybir.AluOpType.add,
                        replica_groups=[list(range(NC))],
                        ins=[po[:].rearrange("p (a b) -> (p a) b", a=NC)],
                        outs=[rso[:]],
                    )
                rs_sb = work.tile([1, HS], f32)
                nc.sync.dma_start(out=rs_sb[:], in_=rso[:])
                ob = work.tile([1, HS], f32)
                nc.scalar.activation(ob[:], rs_sb[:], TANH)
                nc.sync.dma_start(out=out.ap(), in_=ob[:])

            zz_t = const.tile([1, REV], f32)
            nc.sync.dma_start(out=zz_t[:], in_=zz.ap())

    nc.compile()
    return nc


def _prep_in_maps(x, h0_fwd, c0_fwd, h0_rev, c0_rev,
                  Wih_f, Whh_f, bih_f, bhh_f,
                  Wih_r, Whh_r, bih_r, bhh_r,
                  W_lin, b_lin):
    f32 = np.float32
    S = float(2.0 ** SB)
    nt = np.asarray(x, f32)[-1, 0]                               # (2048,)
    h0 = [np.asarray(h0_fwd, f32)[0], np.asarray(h0_rev, f32)[0]]
    c0 = [np.asarray(c0_fwd, f32)[0], np.asarray(c0_rev, f32)[0]]
    Wih = [np.asarray(Wih_f, f32), np.asarray(Wih_r, f32)]       # (8192, 2048)
    Whh = [np.asarray(Whh_f, f32), np.asarray(Whh_r, f32)]
    bg = [np.asarray(bih_f, f32) + np.asarray(bhh_f, f32),
          np.asarray(bih_r, f32) + np.asarray(bhh_r, f32)]
    WT = np.asarray(W_lin, f32).T                                # (4096, 2048)
    bl = np.asarray(b_lin, f32)

    # x lhsT: (128, NXC, 2 dirs-of-W, 2 cols); dir d uses column d
    sb1 = np.zeros((128, NXC, 2, 2), BF)
    ntc = nt.reshape(NXC, 128).T.astype(BF)                      # (128, 16)
    sb1[:, :, 0, 0] = ntc
    sb1[:, :, 1, 1] = ntc

    # h0 DoubleRow lhsT: (128, 2 dirs, NHP, 2 kt, 16): col d = h0_d, 16B pad
    h5 = np.zeros((128, 2, NHP, 2, 16), E5)
    for d in range(2):
        hc = (h0[d] * (2.0 ** SH5)).reshape(NHP, 2, 128)
        h5[:, d, :, :, d] = hc.transpose(2, 0, 1).astype(E5)

    in_maps = []
    for k in range(NC):
        sl = np.arange(k * HS, (k + 1) * HS)
        rowsel = np.concatenate([g * H + sl for g in (0, 1, 3, 2)])  # i,f,o,g
        sb2 = np.zeros((2, 3336), BF)
        wxb_l, wxq_l, wh_l = [], [], []
        for d in range(2):
            WxT = Wih[d][rowsel].T * S                           # (2048, 1024)
            wx = WxT.reshape(NXC, 128, G).transpose(1, 0, 2)     # (128,16,1024)
            if NXB:
                wxb_l.append(np.ascontiguousarray(wx[:, :NXB]).astype(BF))
            wxq_l.append(np.ascontiguousarray(wx[:, NXB:]).astype(E3))
            WhT = Whh[d][rowsel].T * (2.0 ** (SB - SH5))
            whp = WhT.reshape(NHP, 2, 128, G).transpose(2, 0, 1, 3)
            wh_l.append(np.ascontiguousarray(whp).astype(E5))
            sb2[d, 2:2 + G] = (bg[d][rowsel] * S).astype(BF)
        sb2[:, 1026:1282] = np.stack([c0[0][sl], c0[1][sl]]).astype(BF)
        sb2[:, 0:2] = np.eye(2, dtype=BF)
        if k == 0:
            sb2[0, 1282:3330] = bl.astype(BF)
        # W_lin rows for local h, k-chunk order [f0, f1, r0, r1]
        rows = np.concatenate([sl[:128], sl[128:], H + sl[:128], H + sl[128:]])
        wlk = WT[rows].reshape(4, 128, H).transpose(1, 0, 2)
        im = ({"wxb0": wxb_l[0], "wxb1": wxb_l[1]} if NXB else {})
        im.update({
            "sb1": sb1, "sb2": sb2, "h5": h5,
            "wxq0": wxq_l[0], "wxq1": wxq_l[1],
            "wh0": wh_l[0], "wh1": wh_l[1],
            "wl": np.ascontiguousarray(wlk).astype(BF),
            "zz": np.zeros((1, REV), f32),
        })
        in_maps.append(im)
    return in_maps


def kernel(**inputs) -> np.ndarray:
    if "nc" not in _CACHE:
        _CACHE["nc"] = _build_program("full")
    nc = _CACHE["nc"]
    in_maps = _prep_in_maps(**inputs)
    res = run_bass_kernel_spmd(nc, in_maps, core_ids=list(range(NC)))
    return np.concatenate(
        [res.results[k]["out"][0] for k in range(NC)]
    )[None, :].astype(np.float32)
